# revision 36
# baseline (speedup 1.0000x reference)
"""CXLoss kernel for 8 Trainium2 NeuronCores (v5).

Math (per sample n):
  meanT = featureT.mean(axis=(0,2,3))                      (global over batch)
  fT = normalize(featureT[n] - meanT), fI = normalize(featureI[n] - meanT)
  S[q,p] = fI[:,q] . fT[:,p]    (C=256 contraction; p,q in [0,4096))
  raw = (1-S)/2 ; div[q] = min_p raw ; W = exp((1 - raw/(div+eps))/sigma)
  CX = W / (sum_p W + eps) ; out[p] = max_q CX ; loss = mean_n -log(mean_p out + eps)

Sharding: core k handles sample n=k//2 and half of the q axis (h=k%2).
All per-q reductions (over the full p axis) are core-local, so there is
NO cross-core communication on device (no collectives -> no cross-core
sync point; each core's execution window is its own compute only).

Host-side work is limited to input sharding/layout plus two tiny
vector-level steps that bracket the device program:
  - negm = -featureT.mean((0,2,3)) ([256] floats), passed per-core as a
    1KB side input so each core need not re-read all 4 samples (16MB)
    just to reproduce a global 256-float statistic.
  - final combine: elementwise max of core-pair outputs (8x[128,32]) and
    the -log(mean) epilogue.

Dataflow per q-tile (128 q rows x 4096 p):
  TensorE : S_psum = fIc^T @ fTc          (f16 operands, centered only)
  DVE     : ttr evac: s16 = S_psum * rtb  (rt[p] broadcast fold) with a
            chained max-accumulate -> smax_raw (no separate reduce)
  ACT     : divp/scl/gam stats, W = exp(scl*s16+gam) (+wsum accum), wse
  DVE     : (one iteration behind) invw = 1/wse ; cx = W*invw ;
            R = max(R, cx)
Final: transpose R 128-col blocks (TensorE) + grouped psum reduces.
"""

import sys

sys.path.insert(0, "/opt/trn_rl_repo")

import numpy as np
from contextlib import ExitStack

EPS = 1e-8
SIGMA = 0.1
IS = 1.0 / (SIGMA + EPS)  # inverse sigma

N, C, H, W = 4, 256, 64, 64
HW = H * W            # 4096 (p axis; also full q axis)
QH = HW // 2          # 2048 q per core
P128 = 128
C2 = C // P128        # 2 channel chunks
QT = QH // P128       # 16 q tiles
NCOLS = HW // P128    # 32 output columns
NEG_INF = -3.0e38

_CACHE = {}


def _build_nc():
    from concourse import bacc, mybir, masks
    from concourse import tile as tile_mod

    f32 = mybir.dt.float32
    f16 = mybir.dt.float16
    AF = mybir.ActivationFunctionType
    OP = mybir.AluOpType
    AX = mybir.AxisListType

    nc = bacc.Bacc(
        "TRN2",
        target_bir_lowering=False,
        debug=False,
        num_devices=8,
    )

    fT_d = nc.dram_tensor("ft", [C2, P128, HW], f32, kind="ExternalInput").ap()
    fI_d = nc.dram_tensor("fi", [C2, P128, QH], f32, kind="ExternalInput").ap()
    nm_d = nc.dram_tensor("nm", [C2, P128, 1], f32, kind="ExternalInput").ap()
    out_d = nc.dram_tensor("cxo", [P128, NCOLS], f32, kind="ExternalOutput").ap()
    srT_d = nc.dram_tensor("srt", [1, HW], f32).ap()
    srI_d = nc.dram_tensor("sri", [1, QH], f32).ap()

    with tile_mod.TileContext(nc) as tc, ExitStack() as ctx:
        persist = ctx.enter_context(tc.tile_pool(name="persist", bufs=1))

        # persistent matmul operands + per-q stats + constants
        fTn = [persist.tile([P128, HW], f16, name=f"ftn{c}", tag=f"ftn{c}") for c in range(C2)]
        fIc = [persist.tile([P128, QH], f16, name=f"fic{c}", tag=f"fic{c}") for c in range(C2)]
        ri = persist.tile([P128, QT], f32, name="ri", tag="ri")  # 1/(||fI||+eps)

        ones_col = persist.tile([P128, 1], f16, name="ones_col", tag="ones_col")
        ones_row = persist.tile([1, P128], f16, name="ones_row", tag="ones_row")
        id16 = persist.tile([P128, P128], f16, name="id16", tag="id16")
        negm = [persist.tile([P128, 1], f32, name=f"negm{c}", tag=f"negm{c}") for c in range(C2)]
        c_half = persist.tile([P128, 1], f32, name="c_half", tag="c_half")
        c_is = persist.tile([P128, 1], f32, name="c_is", tag="c_is")
        c_eps = persist.tile([P128, 1], f32, name="c_eps", tag="c_eps")
        nc.any.memset(ones_col[:], 1.0)
        nc.any.memset(ones_row[:], 1.0)
        nc.any.memset(c_half[:], 0.5 + EPS)
        nc.any.memset(c_is[:], IS)
        nc.any.memset(c_eps[:], EPS)
        masks.make_identity(nc, id16[:])
        for c in range(C2):
            nc.sync.dma_start(out=negm[c][:], in_=nm_d[c])

        # ---------- load + preprocess ----------
        with ExitStack() as pctx:
            pre = pctx.enter_context(tc.tile_pool(name="pre", bufs=1))
            pps = pctx.enter_context(tc.tile_pool(name="pps", bufs=1, space="PSUM"))
            rps = pctx.enter_context(tc.tile_pool(name="rps", bufs=4, space="PSUM"))

            fT_raw = [pre.tile([P128, HW], f32, name=f"ftraw{c}", tag=f"ftraw{c}") for c in range(C2)]
            fI_raw = [pre.tile([P128, QH], f32, name=f"firaw{c}", tag=f"firaw{c}") for c in range(C2)]
            sqT = [pre.tile([P128, HW], f16, name=f"sqt{c}", tag=f"sqt{c}") for c in range(C2)]
            sqI = [pre.tile([P128, QH], f16, name=f"sqi{c}", tag=f"sqi{c}") for c in range(C2)]

            # fT first: rt (global over p) gates fTn and hence the whole
            # loop, so its inputs must land earliest
            NJ = 4
            for c in range(C2):
                for j in range(NJ):
                    sl = slice(j * HW // NJ, (j + 1) * HW // NJ)
                    nc.sync.dma_start(out=fT_raw[c][:, sl], in_=fT_d[c][:, sl])
                    nc.scalar.activation(
                        sqT[c][:, sl], fT_raw[c][:, sl], AF.Square,
                        bias=negm[c][:], scale=1.0,
                    )
            for c in range(C2):
                for j in range(2):
                    sl = slice(j * QH // 2, (j + 1) * QH // 2)
                    nc.sync.dma_start(out=fI_raw[c][:, sl], in_=fI_d[c][:, sl])
                    nc.scalar.activation(
                        sqI[c][:, sl], fI_raw[c][:, sl], AF.Square,
                        bias=negm[c][:], scale=1.0,
                    )
                    nc.scalar.activation(
                        fIc[c][:, sl], fI_raw[c][:, sl], AF.Identity,
                        bias=negm[c][:], scale=1.0,
                    )

            # ---------- ssq rows via ones-column lhsT (sum over channels) ----
            # out[0, x] = sum_c sq[c, x]  for a 512-wide chunk
            srowT = pre.tile([1, HW], f32, name="srowt", tag="srowt")
            srowI = pre.tile([1, QH], f32, name="srowi", tag="srowi")
            for j in range(8):
                rp = rps.tile([1, 512], f32, name="ssqr", tag="ssqr")
                for kc in range(C2):
                    nc.tensor.matmul(
                        rp[:],
                        lhsT=ones_col[:],
                        rhs=sqT[kc][:, j * 512:(j + 1) * 512],
                        start=(kc == 0),
                        stop=(kc == C2 - 1),
                    )
                eng = nc.scalar if j % 2 == 0 else nc.vector
                if j % 2 == 0:
                    nc.scalar.activation(
                        srowT[:, j * 512:(j + 1) * 512], rp[:], AF.Identity
                    )
                else:
                    nc.vector.tensor_copy(srowT[:, j * 512:(j + 1) * 512], rp[:])
            for j in range(4):
                rp = rps.tile([1, 512], f32, name="ssqr", tag="ssqr")
                for kc in range(C2):
                    nc.tensor.matmul(
                        rp[:],
                        lhsT=ones_col[:],
                        rhs=sqI[kc][:, j * 512:(j + 1) * 512],
                        start=(kc == 0),
                        stop=(kc == C2 - 1),
                    )
                if j % 2 == 0:
                    nc.scalar.activation(
                        srowI[:, j * 512:(j + 1) * 512], rp[:], AF.Identity
                    )
                else:
                    nc.vector.tensor_copy(srowI[:, j * 512:(j + 1) * 512], rp[:])

            # gather rows into column layout (x = col*128 + p_row)
            ssqT_col = pre.tile([P128, NCOLS], f32, name="ssqtc", tag="ssqtc")
            ssqI_col = pre.tile([P128, QT], f32, name="ssqic", tag="ssqic")
            # row -> column redistribution via a DRAM scratch round-trip
            # (both directions are plain strided DMAs)
            nc.sync.dma_start(out=srT_d, in_=srowT[:])
            nc.sync.dma_start(out=srI_d, in_=srowI[:])
            nc.sync.dma_start(
                out=ssqT_col[:],
                in_=srT_d.rearrange("o (t p) -> (o p) t", p=P128),
            )
            nc.sync.dma_start(
                out=ssqI_col[:],
                in_=srI_d.rearrange("o (t p) -> (o p) t", p=P128),
            )

            # rt = 1/(sqrt(ssqT)+eps) [128, 32]
            rt1 = pre.tile([P128, NCOLS], f32, name="rt1", tag="rt1")
            rt = pre.tile([P128, NCOLS], f32, name="rt", tag="rt")
            nc.scalar.activation(rt1[:], ssqT_col[:], AF.Sqrt)
            nc.vector.tensor_scalar_add(rt1[:], rt1[:], EPS)
            nc.vector.reciprocal(rt[:], rt1[:])

            # ri stats [128, 16]
            ri1 = pre.tile([P128, QT], f32, name="ri1", tag="ri1")
            nc.scalar.activation(ri1[:], ssqI_col[:], AF.Sqrt)
            nc.vector.tensor_scalar_add(ri1[:], ri1[:], EPS)
            nc.vector.reciprocal(ri[:], ri1[:])

            # transpose rt -> row layout [1, 4096] f16
            rtT_ps = pps.tile([NCOLS, P128], f32, name="rtt", tag="rtt")
            id32 = pre.tile([P128, P128], f32, name="id32", tag="id32")
            masks.make_identity(nc, id32[:])
            nc.tensor.transpose(rtT_ps[:], rt[:], id32[:])
            rtT = pre.tile([NCOLS, P128], f16, name="rtt_sb", tag="rtt_sb")
            nc.scalar.activation(rtT[:], rtT_ps[:], AF.Identity)
            rt_row = pre.tile([1, HW], f16, name="rt_row", tag="rt_row")
            nc.sync.dma_start(
                out=rt_row[:].rearrange("o (t c) -> o t c", c=P128), in_=rtT[:]
            )

            # broadcast rt along partitions; fTn = (fT_raw + negm) * rtb (f16)
            for pc in range(8):
                rtb_ps = pps.tile([P128, 512], f32, name="rtbp", tag="rtbp", bufs=3)
                nc.tensor.matmul(
                    rtb_ps[:],
                    lhsT=ones_row[:],
                    rhs=rt_row[:, pc * 512:(pc + 1) * 512],
                    start=True,
                    stop=True,
                )
                sl = slice(pc * 512, (pc + 1) * 512)
                for c in range(C2):
                    nc.vector.scalar_tensor_tensor(
                        out=fTn[c][:, sl],
                        in0=fT_raw[c][:, sl],
                        scalar=negm[c][:],
                        in1=rtb_ps[:],
                        op0=OP.add,
                        op1=OP.mult,
                    )

        # ---------- main loop over q tiles ----------
        rpool = ctx.enter_context(tc.tile_pool(name="rpool", bufs=2))
        r_prev = rpool.tile([P128, HW], f16, name="R", tag="R")
        nc.any.memset(r_prev[:], 0.0)

        with ExitStack() as fctx:
            fin = fctx.enter_context(tc.tile_pool(name="fin", bufs=1))
            cxm = fin.tile([P128, NCOLS], f32, name="cxm", tag="cxm")
            mctx = ExitStack()
            mm = mctx.enter_context(tc.tile_pool(name="mm", bufs=2))
            st = mctx.enter_context(tc.tile_pool(name="st", bufs=4))
            sps = mctx.enter_context(tc.tile_pool(name="sps", bufs=1, space="PSUM"))

            ps = [sps.tile([P128, 1024], f32, name=f"ps{j}", tag=f"ps{j}") for j in range(4)]

            def emit_r_update(pend):
                """Deferred R-update: cx = wt*invw ; R = max(R, cx). Runs
                one iteration behind so DVE never waits on that tile's
                exp."""
                nonlocal r_prev
                wt_p, wse_p = pend
                invw = st.tile([P128, 1], f32, name="invw", tag="invw")
                nc.vector.reciprocal(invw[:], wse_p[:])
                cx = mm.tile([P128, HW], f16, name="cx", tag="cx")
                r_new = rpool.tile([P128, HW], f16, name="R", tag="R")
                nc.vector.tensor_scalar(
                    out=cx[:], in0=wt_p[:], scalar1=invw[:],
                    scalar2=None, op0=OP.mult,
                )
                nc.vector.tensor_max(r_new[:], cx[:], r_prev[:])
                r_prev = r_new

            pending = None
            for t in range(QT):
                tq = slice(t * P128, (t + 1) * P128)
                s16 = mm.tile([P128, HW], f16, name="s16", tag="s16")
                smx = st.tile([P128, 4], f32, name="smx", tag="smx")

                for kc in range(C2):
                    for j in range(4):
                        for h in range(2):
                            sl_p = slice((2 * j + h) * 512, (2 * j + h + 1) * 512)
                            nc.tensor.matmul(
                                ps[j][:, h * 512:(h + 1) * 512],
                                lhsT=fIc[kc][:, tq],
                                rhs=fTn[kc][:, sl_p],
                                start=(kc == 0),
                                stop=(kc == C2 - 1),
                            )

                # evacuate PSUM -> f16 SBUF on DVE, folding the ri[q]
                # scale (s16 = S_true) and a fused running max over p
                for j in range(4):
                    sl = slice(j * 1024, (j + 1) * 1024)
                    nc.vector.tensor_scalar(
                        out=s16[:, sl],
                        in0=ps[j][:],
                        scalar1=ri[:, t:t + 1],
                        scalar2=None,
                        op0=OP.mult,
                        op1=OP.max,
                        accum_out=smx[:, j:j + 1],
                    )
                smax_raw = st.tile([P128, 1], f32, name="smaxr", tag="smaxr")
                nc.vector.reduce_max(out=smax_raw[:], in_=smx[:], axis=AX.X)

                # stats: divp = div+eps = (1-smax)/2 + eps ; rdiv = 1/divp ;
                # scl = (IS/2)*rdiv ; gam = IS - (IS/2)*rdiv
                divp = st.tile([P128, 1], f32, name="divp", tag="divp")
                rdiv = st.tile([P128, 1], f32, name="rdiv", tag="rdiv")
                scl = st.tile([P128, 1], f32, name="scl", tag="scl")
                gam = st.tile([P128, 1], f32, name="gam", tag="gam")
                nc.scalar.activation(
                    divp[:], smax_raw[:], AF.Identity,
                    bias=c_half[:], scale=-0.5,
                )
                nc.vector.reciprocal(rdiv[:], divp[:])
                nc.scalar.activation(
                    scl[:], rdiv[:], AF.Identity, scale=IS / 2.0
                )
                nc.scalar.activation(
                    gam[:], rdiv[:], AF.Identity, bias=c_is[:], scale=-IS / 2.0
                )

                # W = exp(scl*S + gam), wsum = sum_p W  (ACT)
                # (last tile's W/wse live in the fin pool: they are read
                # after the main-loop pools close)
                last = t == QT - 1
                wt = (fin if last else mm).tile([P128, HW], f16, name="wt", tag="wtf" if last else "wt")
                wsum = st.tile([P128, 1], f32, name="wsum", tag="wsum")
                nc.scalar.activation(
                    wt[:], s16[:], AF.Exp, bias=gam[:], scale=scl[:],
                    accum_out=wsum[:],
                )
                wse = (fin if last else st).tile([P128, 1], f32, name="wse", tag="wsef" if last else "wse")
                nc.scalar.activation(wse[:], wsum[:], AF.Identity, bias=c_eps[:])

                if pending is not None:
                    emit_r_update(pending)
                pending = (wt, wse)

            # final tile: R-update chunked, interleaved with the
            # transpose+reduce output stage (main-loop PSUM is released
            # first so the transposes have banks to land in)
            wt_f, wse_f = pending
            invw_f = fin.tile([P128, 1], f32, name="invwf", tag="invwf")
            nc.vector.reciprocal(invw_f[:], wse_f[:])
            cx_f = fin.tile([P128, HW], f16, name="cxf", tag="cxf")
            r_fin = rpool.tile([P128, HW], f16, name="R", tag="R")
            mctx.close()
            fps = fctx.enter_context(tc.tile_pool(name="fps", bufs=4, space="PSUM"))
            for jc in range(4):
                sl = slice(jc * 1024, (jc + 1) * 1024)
                nc.vector.tensor_scalar(
                    out=cx_f[:, sl], in0=wt_f[:, sl], scalar1=invw_f[:],
                    scalar2=None, op0=OP.mult,
                )
                nc.vector.tensor_max(r_fin[:, sl], cx_f[:, sl], r_prev[:, sl])
                for g in range(2):
                    tp = fps.tile([P128, 4 * P128], f16, name="tp", tag="tp")
                    for u in range(4):
                        cc = jc * 8 + g * 4 + u
                        nc.tensor.transpose(
                            tp[:, u * P128:(u + 1) * P128],
                            r_fin[:, cc * P128:(cc + 1) * P128],
                            id16[:],
                        )
                    gc = jc * 8 + g * 4
                    nc.vector.reduce_max(
                        out=cxm[:, gc:gc + 4],
                        in_=tp[:].rearrange("p (u c) -> p u c", c=P128),
                        axis=AX.X,
                    )

            nc.sync.dma_start(out=out_d[:], in_=cxm[:])

    nc.compile()
    return nc


def _get_nc():
    if "nc" not in _CACHE:
        _CACHE["nc"] = _build_nc()
    return _CACHE["nc"]


def _make_in_maps(featureT, featureI):
    featureT = np.asarray(featureT, dtype=np.float32)
    featureI = np.asarray(featureI, dtype=np.float32)
    negm = (-featureT.mean(axis=(0, 2, 3))).astype(np.float32).reshape(C2, P128, 1)
    negm = np.ascontiguousarray(negm)
    in_maps = []
    for k in range(8):
        n, h = k // 2, k % 2
        ft = np.ascontiguousarray(featureT[n].reshape(C2, P128, HW))
        fi = np.ascontiguousarray(
            featureI[n].reshape(C, HW)[:, h * QH:(h + 1) * QH].reshape(C2, P128, QH)
        )
        in_maps.append({"ft": ft, "fi": fi, "nm": negm})
    return in_maps


def _ensure_ntff_hook():
    """If tracing is requested in an environment whose antenv lacks
    axon_hooks, synthesize the module and register the ctypes NTFF hook
    so run_bass_kernel_spmd's trace path works instead of crashing."""
    import os
    import types

    try:
        from antenv import axon_hooks  # noqa: F401
        return
    except ImportError:
        pass
    try:
        mod = types.ModuleType("antenv.axon_hooks")
        _state = {"hook": None}
        mod.set_axon_ntff_profile_hook = lambda h: _state.__setitem__("hook", h)
        mod.get_axon_ntff_profile_hook = lambda: _state["hook"]
        sys.modules["antenv.axon_hooks"] = mod
        import antenv

        antenv.axon_hooks = mod
        from trn_agent_boot.trn_boot import _ntff_profile_via_ctypes

        so = "/opt/axon/libaxon_pjrt.so"
        if os.path.exists(so):
            mod.set_axon_ntff_profile_hook(_ntff_profile_via_ctypes(so))
    except Exception:
        pass


def run(featureT, featureI, trace=False):
    from concourse.bass_utils import run_bass_kernel_spmd

    _ensure_ntff_hook()
    nc = _get_nc()
    in_maps = _make_in_maps(featureT, featureI)
    res = run_bass_kernel_spmd(nc, in_maps, list(range(8)), trace=trace)
    outs = [np.asarray(res.results[k]["cxo"], dtype=np.float64) for k in range(8)]
    losses = []
    for n in range(N):
        cx = np.maximum(outs[2 * n], outs[2 * n + 1])
        losses.append(-np.log(cx.mean() + EPS))
    loss = np.float32(np.mean(losses))
    return loss, res


def kernel(featureT, featureI):
    loss, _ = run(featureT, featureI, trace=False)
    return loss


# revision 42
# speedup vs baseline: 1.2454x; 1.2454x over previous
"""CXLoss kernel for 8 Trainium2 NeuronCores (v5).

Math (per sample n):
  meanT = featureT.mean(axis=(0,2,3))                      (global over batch)
  fT = normalize(featureT[n] - meanT), fI = normalize(featureI[n] - meanT)
  S[q,p] = fI[:,q] . fT[:,p]    (C=256 contraction; p,q in [0,4096))
  raw = (1-S)/2 ; div[q] = min_p raw ; W = exp((1 - raw/(div+eps))/sigma)
  CX = W / (sum_p W + eps) ; out[p] = max_q CX ; loss = mean_n -log(mean_p out + eps)

Sharding: core k handles sample n=k//2 and half of the q axis (h=k%2).
All per-q reductions (over the full p axis) are core-local, so there is
NO cross-core communication on device (no collectives -> no cross-core
sync point; each core's execution window is its own compute only).

Host-side work is limited to input sharding/layout plus two tiny
vector-level steps that bracket the device program:
  - negm = -featureT.mean((0,2,3)) ([256] floats), passed per-core as a
    1KB side input so each core need not re-read all 4 samples (16MB)
    just to reproduce a global 256-float statistic.
  - final combine: elementwise max of core-pair outputs (8x[128,32]) and
    the -log(mean) epilogue.

Dataflow per q-tile (128 q rows x 4096 p):
  TensorE : S_psum = fIc^T @ fTn    (f16 operands; fTn is centered and
            pre-scaled by the per-p norm reciprocal rt[p])
  DVE     : evac: s16 = S_psum * ri[q] (= S_true, f16) with a fused
            max-accumulate per chunk -> smax via one tiny reduce
  ACT     : divp/scl/gam stats, W = exp(scl*s16+gam) (+wsum accum), wse
  DVE     : (one iteration behind, so DVE never waits on exp)
            invw = 1/wse ; cx = W*invw (4x) ; R = max(R, cx) (2x)
Final: transpose R 128-col blocks (TensorE) + grouped psum reduces.

Notes from HW bring-up: tensor_tensor_reduce and Pool-engine tensor ops
pass CoreSim+compile but crash TRN2 hardware here, so the evacuation
uses tensor_scalar(accum_out, op1=max) and everything heavy stays on
DVE/ACT.
"""

import sys

sys.path.insert(0, "/opt/trn_rl_repo")

import numpy as np
from contextlib import ExitStack

EPS = 1e-8
SIGMA = 0.1
IS = 1.0 / (SIGMA + EPS)  # inverse sigma

N, C, H, W = 4, 256, 64, 64
HW = H * W            # 4096 (p axis; also full q axis)
QH = HW // 2          # 2048 q per core
P128 = 128
C2 = C // P128        # 2 channel chunks
QT = QH // P128       # 16 q tiles
NCOLS = HW // P128    # 32 output columns
NEG_INF = -3.0e38

_CACHE = {}


def _build_nc():
    from concourse import bacc, mybir, masks
    from concourse import tile as tile_mod

    f32 = mybir.dt.float32
    f16 = mybir.dt.float16
    AF = mybir.ActivationFunctionType
    OP = mybir.AluOpType
    AX = mybir.AxisListType

    nc = bacc.Bacc(
        "TRN2",
        target_bir_lowering=False,
        debug=False,
        num_devices=8,
    )

    fT_d = nc.dram_tensor("ft", [C2, P128, HW], f32, kind="ExternalInput").ap()
    fI_d = nc.dram_tensor("fi", [C2, P128, QH], f32, kind="ExternalInput").ap()
    nm_d = nc.dram_tensor("nm", [C2, P128, 1], f32, kind="ExternalInput").ap()
    out_d = nc.dram_tensor("cxo", [P128, NCOLS], f32, kind="ExternalOutput").ap()

    with tile_mod.TileContext(nc) as tc, ExitStack() as ctx:
        persist = ctx.enter_context(tc.tile_pool(name="persist", bufs=1))

        # persistent matmul operands + per-q stats + constants
        fTn = [persist.tile([P128, HW], f16, name=f"ftn{c}", tag=f"ftn{c}") for c in range(C2)]
        fIc = [persist.tile([P128, QH], f16, name=f"fic{c}", tag=f"fic{c}") for c in range(C2)]
        ri = persist.tile([P128, QT], f32, name="ri", tag="ri")  # 1/(||fI||+eps)

        ones_col = persist.tile([P128, 1], f16, name="ones_col", tag="ones_col")
        ones_row = persist.tile([1, P128], f16, name="ones_row", tag="ones_row")
        id16 = persist.tile([P128, P128], f16, name="id16", tag="id16")
        negm = [persist.tile([P128, 1], f32, name=f"negm{c}", tag=f"negm{c}") for c in range(C2)]
        c_half = persist.tile([P128, 1], f32, name="c_half", tag="c_half")
        c_is = persist.tile([P128, 1], f32, name="c_is", tag="c_is")
        c_eps = persist.tile([P128, 1], f32, name="c_eps", tag="c_eps")
        nc.any.memset(ones_col[:], 1.0)
        nc.any.memset(ones_row[:], 1.0)
        nc.any.memset(c_half[:], 0.5 + EPS)
        nc.any.memset(c_is[:], IS)
        nc.any.memset(c_eps[:], EPS)
        masks.make_identity(nc, id16[:])
        for c in range(C2):
            nc.sync.dma_start(out=negm[c][:], in_=nm_d[c])

        # ---------- load + preprocess ----------
        with ExitStack() as pctx:
            pre = pctx.enter_context(tc.tile_pool(name="pre", bufs=1))
            pps = pctx.enter_context(tc.tile_pool(name="pps", bufs=1, space="PSUM"))

            fT_raw = [pre.tile([P128, HW], f32, name=f"ftraw{c}", tag=f"ftraw{c}") for c in range(C2)]
            fI_raw = [pre.tile([P128, QH], f32, name=f"firaw{c}", tag=f"firaw{c}") for c in range(C2)]
            sqT = [pre.tile([P128, HW], f16, name=f"sqt{c}", tag=f"sqt{c}") for c in range(C2)]
            sqI = [pre.tile([P128, QH], f16, name=f"sqi{c}", tag=f"sqi{c}") for c in range(C2)]

            # fT first: rt (global over p) gates fTn and hence the whole
            # loop, so its inputs must land earliest
            NJ = 4
            for c in range(C2):
                for j in range(NJ):
                    sl = slice(j * HW // NJ, (j + 1) * HW // NJ)
                    nc.sync.dma_start(out=fT_raw[c][:, sl], in_=fT_d[c][:, sl])
                    nc.scalar.activation(
                        sqT[c][:, sl], fT_raw[c][:, sl], AF.Square,
                        bias=negm[c][:], scale=1.0,
                    )
            for c in range(C2):
                for j in range(2):
                    sl = slice(j * QH // 2, (j + 1) * QH // 2)
                    nc.sync.dma_start(out=fI_raw[c][:, sl], in_=fI_d[c][:, sl])
                    nc.scalar.activation(
                        sqI[c][:, sl], fI_raw[c][:, sl], AF.Square,
                        bias=negm[c][:], scale=1.0,
                    )
                    nc.scalar.activation(
                        fIc[c][:, sl], fI_raw[c][:, sl], AF.Identity,
                        bias=negm[c][:], scale=1.0,
                    )

            # ---------- ssq columns via per-128-chunk gram with ones ----------
            # out[i, t] = sum_c sq[c, t*128+i]
            ssqT_ps = pps.tile([P128, NCOLS], f32, name="ssqt", tag="ssqt")
            ssqI_ps = pps.tile([P128, QT], f32, name="ssqi", tag="ssqi")
            for t in range(NCOLS):
                for kc in range(C2):
                    nc.tensor.matmul(
                        ssqT_ps[:, t:t + 1],
                        lhsT=sqT[kc][:, t * P128:(t + 1) * P128],
                        rhs=ones_col[:],
                        start=(kc == 0),
                        stop=(kc == C2 - 1),
                    )
            for t in range(QT):
                for kc in range(C2):
                    nc.tensor.matmul(
                        ssqI_ps[:, t:t + 1],
                        lhsT=sqI[kc][:, t * P128:(t + 1) * P128],
                        rhs=ones_col[:],
                        start=(kc == 0),
                        stop=(kc == C2 - 1),
                    )

            # rt = 1/(sqrt(ssqT)+eps) [128, 32]
            rt1 = pre.tile([P128, NCOLS], f32, name="rt1", tag="rt1")
            rt = pre.tile([P128, NCOLS], f32, name="rt", tag="rt")
            nc.scalar.activation(rt1[:], ssqT_ps[:], AF.Sqrt)
            nc.vector.tensor_scalar_add(rt1[:], rt1[:], EPS)
            nc.vector.reciprocal(rt[:], rt1[:])

            # ri stats [128, 16]
            ri1 = pre.tile([P128, QT], f32, name="ri1", tag="ri1")
            nc.scalar.activation(ri1[:], ssqI_ps[:], AF.Sqrt)
            nc.vector.tensor_scalar_add(ri1[:], ri1[:], EPS)
            nc.vector.reciprocal(ri[:], ri1[:])

            # transpose rt -> row layout [1, 4096] f16
            rtT_ps = pps.tile([NCOLS, P128], f32, name="rtt", tag="rtt")
            id32 = pre.tile([P128, P128], f32, name="id32", tag="id32")
            masks.make_identity(nc, id32[:])
            nc.tensor.transpose(rtT_ps[:], rt[:], id32[:])
            rtT = pre.tile([NCOLS, P128], f16, name="rtt_sb", tag="rtt_sb")
            nc.scalar.activation(rtT[:], rtT_ps[:], AF.Identity)
            rt_row = pre.tile([1, HW], f16, name="rt_row", tag="rt_row")
            nc.sync.dma_start(
                out=rt_row[:].rearrange("o (t c) -> o t c", c=P128), in_=rtT[:]
            )

            # broadcast rt along partitions; fTn = (fT_raw + negm) * rtb (f16)
            for pc in range(8):
                rtb_ps = pps.tile([P128, 512], f32, name="rtbp", tag="rtbp", bufs=3)
                nc.tensor.matmul(
                    rtb_ps[:],
                    lhsT=ones_row[:],
                    rhs=rt_row[:, pc * 512:(pc + 1) * 512],
                    start=True,
                    stop=True,
                )
                sl = slice(pc * 512, (pc + 1) * 512)
                for c in range(C2):
                    nc.vector.scalar_tensor_tensor(
                        out=fTn[c][:, sl],
                        in0=fT_raw[c][:, sl],
                        scalar=negm[c][:],
                        in1=rtb_ps[:],
                        op0=OP.add,
                        op1=OP.mult,
                    )

        # ---------- main loop over q tiles ----------
        rpool = ctx.enter_context(tc.tile_pool(name="rpool", bufs=2))
        r_prev = rpool.tile([P128, HW], f16, name="R", tag="R")
        nc.any.memset(r_prev[:], 0.0)

        with ExitStack() as mctx:
            mm = mctx.enter_context(tc.tile_pool(name="mm", bufs=2))
            st = mctx.enter_context(tc.tile_pool(name="st", bufs=4))
            sps = mctx.enter_context(tc.tile_pool(name="sps", bufs=1, space="PSUM"))

            ps = [sps.tile([P128, 1024], f32, name=f"ps{j}", tag=f"ps{j}") for j in range(4)]

            def emit_r_update(pend):
                """Deferred R-update: cx = wt*invw ; R = max(R, cx). Runs
                one iteration behind so DVE never waits on that tile's
                exp."""
                nonlocal r_prev
                wt_p, wse_p = pend
                invw = st.tile([P128, 1], f32, name="invw", tag="invw")
                nc.vector.reciprocal(invw[:], wse_p[:])
                cx = mm.tile([P128, HW], f16, name="cx", tag="cx")
                r_new = rpool.tile([P128, HW], f16, name="R", tag="R")
                nc.vector.tensor_scalar(
                    out=cx[:], in0=wt_p[:], scalar1=invw[:],
                    scalar2=None, op0=OP.mult,
                )
                nc.vector.tensor_max(r_new[:], cx[:], r_prev[:])
                r_prev = r_new

            pending = None
            for t in range(QT):
                tq = slice(t * P128, (t + 1) * P128)
                s16 = mm.tile([P128, HW], f16, name="s16", tag="s16")
                smx = st.tile([P128, 4], f32, name="smx", tag="smx")

                for kc in range(C2):
                    for j in range(4):
                        for h in range(2):
                            sl_p = slice((2 * j + h) * 512, (2 * j + h + 1) * 512)
                            nc.tensor.matmul(
                                ps[j][:, h * 512:(h + 1) * 512],
                                lhsT=fIc[kc][:, tq],
                                rhs=fTn[kc][:, sl_p],
                                start=(kc == 0),
                                stop=(kc == C2 - 1),
                            )

                # evacuate PSUM -> f16 SBUF on DVE, folding the ri[q]
                # scale (s16 = S_true) and a fused running max over p
                for j in range(4):
                    sl = slice(j * 1024, (j + 1) * 1024)
                    nc.vector.tensor_scalar(
                        out=s16[:, sl],
                        in0=ps[j][:],
                        scalar1=ri[:, t:t + 1],
                        scalar2=None,
                        op0=OP.mult,
                        op1=OP.max,
                        accum_out=smx[:, j:j + 1],
                    )
                smax_raw = st.tile([P128, 1], f32, name="smaxr", tag="smaxr")
                nc.vector.reduce_max(out=smax_raw[:], in_=smx[:], axis=AX.X)

                # stats: divp = div+eps = (1-smax)/2 + eps ; rdiv = 1/divp ;
                # scl = (IS/2)*rdiv ; gam = IS - (IS/2)*rdiv
                divp = st.tile([P128, 1], f32, name="divp", tag="divp")
                rdiv = st.tile([P128, 1], f32, name="rdiv", tag="rdiv")
                scl = st.tile([P128, 1], f32, name="scl", tag="scl")
                gam = st.tile([P128, 1], f32, name="gam", tag="gam")
                nc.scalar.activation(
                    divp[:], smax_raw[:], AF.Identity,
                    bias=c_half[:], scale=-0.5,
                )
                nc.vector.reciprocal(rdiv[:], divp[:])
                nc.scalar.activation(
                    scl[:], rdiv[:], AF.Identity, scale=IS / 2.0
                )
                nc.scalar.activation(
                    gam[:], rdiv[:], AF.Identity, bias=c_is[:], scale=-IS / 2.0
                )

                # W = exp(scl*S + gam), wsum = sum_p W  (ACT)
                wt = mm.tile([P128, HW], f16, name="wt", tag="wt")
                wsum = st.tile([P128, 1], f32, name="wsum", tag="wsum")
                nc.scalar.activation(
                    wt[:], s16[:], AF.Exp, bias=gam[:], scale=scl[:],
                    accum_out=wsum[:],
                )
                wse = st.tile([P128, 1], f32, name="wse", tag="wse")
                nc.scalar.activation(wse[:], wsum[:], AF.Identity, bias=c_eps[:])

                if pending is not None:
                    emit_r_update(pending)
                pending = (wt, wse)
            emit_r_update(pending)

        # ---------- final: max over the 128 q-partitions per p ----------
        with ExitStack() as fctx:
            fin = fctx.enter_context(tc.tile_pool(name="fin", bufs=1))
            fps = fctx.enter_context(tc.tile_pool(name="fps", bufs=4, space="PSUM"))
            cxm = fin.tile([P128, NCOLS], f32, name="cxm", tag="cxm")
            for g in range(8):
                tp = fps.tile([P128, 4 * P128], f16, name="tp", tag="tp")
                for u in range(4):
                    cc = g * 4 + u
                    nc.tensor.transpose(
                        tp[:, u * P128:(u + 1) * P128],
                        r_prev[:, cc * P128:(cc + 1) * P128],
                        id16[:],
                    )
                nc.vector.reduce_max(
                    out=cxm[:, g * 4:(g + 1) * 4],
                    in_=tp[:].rearrange("p (u c) -> p u c", c=P128),
                    axis=AX.X,
                )
            nc.sync.dma_start(out=out_d[:], in_=cxm[:])

    nc.compile()
    return nc


def _get_nc():
    if "nc" not in _CACHE:
        _CACHE["nc"] = _build_nc()
    return _CACHE["nc"]


def _make_in_maps(featureT, featureI):
    featureT = np.asarray(featureT, dtype=np.float32)
    featureI = np.asarray(featureI, dtype=np.float32)
    negm = (-featureT.mean(axis=(0, 2, 3))).astype(np.float32).reshape(C2, P128, 1)
    negm = np.ascontiguousarray(negm)
    in_maps = []
    for k in range(8):
        n, h = k // 2, k % 2
        ft = np.ascontiguousarray(featureT[n].reshape(C2, P128, HW))
        fi = np.ascontiguousarray(
            featureI[n].reshape(C, HW)[:, h * QH:(h + 1) * QH].reshape(C2, P128, QH)
        )
        in_maps.append({"ft": ft, "fi": fi, "nm": negm})
    return in_maps


def _ensure_ntff_hook():
    """If tracing is requested in an environment whose antenv lacks
    axon_hooks, synthesize the module and register the ctypes NTFF hook
    so run_bass_kernel_spmd's trace path works instead of crashing."""
    import os
    import types

    try:
        from antenv import axon_hooks  # noqa: F401
        return
    except ImportError:
        pass
    try:
        mod = types.ModuleType("antenv.axon_hooks")
        _state = {"hook": None}
        mod.set_axon_ntff_profile_hook = lambda h: _state.__setitem__("hook", h)
        mod.get_axon_ntff_profile_hook = lambda: _state["hook"]
        sys.modules["antenv.axon_hooks"] = mod
        import antenv

        antenv.axon_hooks = mod
        from trn_agent_boot.trn_boot import _ntff_profile_via_ctypes

        so = "/opt/axon/libaxon_pjrt.so"
        if os.path.exists(so):
            mod.set_axon_ntff_profile_hook(_ntff_profile_via_ctypes(so))
    except Exception:
        pass


def run(featureT, featureI, trace=False):
    from concourse.bass_utils import run_bass_kernel_spmd

    _ensure_ntff_hook()
    nc = _get_nc()
    in_maps = _make_in_maps(featureT, featureI)
    res = run_bass_kernel_spmd(nc, in_maps, list(range(8)), trace=trace)
    outs = [np.asarray(res.results[k]["cxo"], dtype=np.float64) for k in range(8)]
    losses = []
    for n in range(N):
        cx = np.maximum(outs[2 * n], outs[2 * n + 1])
        losses.append(-np.log(cx.mean() + EPS))
    loss = np.float32(np.mean(losses))
    return loss, res


def kernel(featureT, featureI):
    loss, _ = run(featureT, featureI, trace=False)
    return loss


# revision 46
# speedup vs baseline: 1.2598x; 1.0116x over previous
"""CXLoss kernel for 8 Trainium2 NeuronCores (v5).

Math (per sample n):
  meanT = featureT.mean(axis=(0,2,3))                      (global over batch)
  fT = normalize(featureT[n] - meanT), fI = normalize(featureI[n] - meanT)
  S[q,p] = fI[:,q] . fT[:,p]    (C=256 contraction; p,q in [0,4096))
  raw = (1-S)/2 ; div[q] = min_p raw ; W = exp((1 - raw/(div+eps))/sigma)
  CX = W / (sum_p W + eps) ; out[p] = max_q CX ; loss = mean_n -log(mean_p out + eps)

Sharding: core k handles sample n=k//2 and half of the q axis (h=k%2).
All per-q reductions (over the full p axis) are core-local, so there is
NO cross-core communication on device (no collectives -> no cross-core
sync point; each core's execution window is its own compute only).

Host-side work is limited to input sharding/layout plus two tiny
vector-level steps that bracket the device program:
  - negm = -featureT.mean((0,2,3)) ([256] floats), passed per-core as a
    1KB side input so each core need not re-read all 4 samples (16MB)
    just to reproduce a global 256-float statistic.
  - final combine: elementwise max of core-pair outputs (8x[128,32]) and
    the -log(mean) epilogue.

Dataflow per q-tile (128 q rows x 4096 p):
  TensorE : S_psum = fIc^T @ fTn    (f16 operands; fTn is centered and
            pre-scaled by the per-p norm reciprocal rt[p])
  DVE     : evac: s16 = S_psum * ri[q] (= S_true, f16) with a fused
            max-accumulate per chunk -> smax via one tiny reduce
  ACT     : divp/scl/gam stats, W = exp(scl*s16+gam) (+wsum accum), wse
  DVE     : (one iteration behind, so DVE never waits on exp)
            invw = 1/wse ; cx = W*invw (4x) ; R = max(R, cx) (2x)
Final: transpose R 128-col blocks (TensorE) + grouped psum reduces.

Notes from HW bring-up: tensor_tensor_reduce and Pool-engine tensor ops
pass CoreSim+compile but crash TRN2 hardware here, so the evacuation
uses tensor_scalar(accum_out, op1=max) and everything heavy stays on
DVE/ACT.
"""

import sys

sys.path.insert(0, "/opt/trn_rl_repo")

import numpy as np
from contextlib import ExitStack

EPS = 1e-8
SIGMA = 0.1
IS = 1.0 / (SIGMA + EPS)  # inverse sigma

N, C, H, W = 4, 256, 64, 64
HW = H * W            # 4096 (p axis; also full q axis)
QH = HW // 2          # 2048 q per core
P128 = 128
C2 = C // P128        # 2 channel chunks
QT = QH // P128       # 16 q tiles
NCOLS = HW // P128    # 32 output columns
NEG_INF = -3.0e38

_CACHE = {}


def _build_nc():
    from concourse import bacc, mybir, masks
    from concourse import tile as tile_mod

    f32 = mybir.dt.float32
    f16 = mybir.dt.float16
    AF = mybir.ActivationFunctionType
    OP = mybir.AluOpType
    AX = mybir.AxisListType

    nc = bacc.Bacc(
        "TRN2",
        target_bir_lowering=False,
        debug=False,
        num_devices=8,
    )

    fT_d = nc.dram_tensor("ft", [C2, P128, HW], f32, kind="ExternalInput").ap()
    fI_d = nc.dram_tensor("fi", [C2, P128, QH], f32, kind="ExternalInput").ap()
    nm_d = nc.dram_tensor("nm", [C2, P128, 1], f32, kind="ExternalInput").ap()
    out_d = nc.dram_tensor("cxo", [P128, NCOLS], f32, kind="ExternalOutput").ap()

    with tile_mod.TileContext(nc) as tc, ExitStack() as ctx:
        persist = ctx.enter_context(tc.tile_pool(name="persist", bufs=1))

        # persistent matmul operands + per-q stats + constants
        fTn = [persist.tile([P128, HW], f16, name=f"ftn{c}", tag=f"ftn{c}") for c in range(C2)]
        fIc = [persist.tile([P128, QH], f16, name=f"fic{c}", tag=f"fic{c}") for c in range(C2)]
        ri = persist.tile([P128, QT], f32, name="ri", tag="ri")  # 1/(||fI||+eps)

        ones_col = persist.tile([P128, 1], f16, name="ones_col", tag="ones_col")
        ones_row = persist.tile([1, P128], f16, name="ones_row", tag="ones_row")
        id16 = persist.tile([P128, P128], f16, name="id16", tag="id16")
        negm = [persist.tile([P128, 1], f32, name=f"negm{c}", tag=f"negm{c}") for c in range(C2)]
        c_half = persist.tile([P128, 1], f32, name="c_half", tag="c_half")
        c_is = persist.tile([P128, 1], f32, name="c_is", tag="c_is")
        c_eps = persist.tile([P128, 1], f32, name="c_eps", tag="c_eps")
        nc.any.memset(ones_col[:], 1.0)
        nc.any.memset(ones_row[:], 1.0)
        nc.any.memset(c_half[:], 0.5 + EPS)
        nc.any.memset(c_is[:], IS)
        nc.any.memset(c_eps[:], EPS)
        masks.make_identity(nc, id16[:])
        for c in range(C2):
            nc.sync.dma_start(out=negm[c][:], in_=nm_d[c])

        # ---------- load + preprocess ----------
        with ExitStack() as pctx:
            pre = pctx.enter_context(tc.tile_pool(name="pre", bufs=1))
            pps = pctx.enter_context(tc.tile_pool(name="pps", bufs=1, space="PSUM"))

            fT_raw = [pre.tile([P128, HW], f32, name=f"ftraw{c}", tag=f"ftraw{c}") for c in range(C2)]
            fI_raw = [pre.tile([P128, QH], f32, name=f"firaw{c}", tag=f"firaw{c}") for c in range(C2)]
            sqT = [pre.tile([P128, HW], f16, name=f"sqt{c}", tag=f"sqt{c}") for c in range(C2)]
            sqI = [pre.tile([P128, QH], f16, name=f"sqi{c}", tag=f"sqi{c}") for c in range(C2)]

            # fT first: rt (global over p) gates fTn and hence the whole
            # loop, so its inputs must land earliest
            NJ = 4
            for c in range(C2):
                for j in range(NJ):
                    sl = slice(j * HW // NJ, (j + 1) * HW // NJ)
                    nc.sync.dma_start(out=fT_raw[c][:, sl], in_=fT_d[c][:, sl])
                    nc.scalar.activation(
                        sqT[c][:, sl], fT_raw[c][:, sl], AF.Square,
                        bias=negm[c][:], scale=1.0,
                    )
            for c in range(C2):
                for j in range(2):
                    sl = slice(j * QH // 2, (j + 1) * QH // 2)
                    nc.sync.dma_start(out=fI_raw[c][:, sl], in_=fI_d[c][:, sl])
                    nc.scalar.activation(
                        sqI[c][:, sl], fI_raw[c][:, sl], AF.Square,
                        bias=negm[c][:], scale=1.0,
                    )
                    nc.scalar.activation(
                        fIc[c][:, sl], fI_raw[c][:, sl], AF.Identity,
                        bias=negm[c][:], scale=1.0,
                    )

            # ---------- ssq columns via per-128-chunk gram with ones ----------
            # out[i, t] = sum_c sq[c, t*128+i]
            ssqT_ps = pps.tile([P128, NCOLS], f32, name="ssqt", tag="ssqt")
            ssqI_ps = pps.tile([P128, QT], f32, name="ssqi", tag="ssqi")
            for t in range(NCOLS):
                for kc in range(C2):
                    nc.tensor.matmul(
                        ssqT_ps[:, t:t + 1],
                        lhsT=sqT[kc][:, t * P128:(t + 1) * P128],
                        rhs=ones_col[:],
                        start=(kc == 0),
                        stop=(kc == C2 - 1),
                    )
            # rt = 1/(sqrt(ssqT)+eps) [128, 32]
            rt1 = pre.tile([P128, NCOLS], f32, name="rt1", tag="rt1")
            rt = pre.tile([P128, NCOLS], f32, name="rt", tag="rt")
            nc.scalar.activation(rt1[:], ssqT_ps[:], AF.Sqrt)
            nc.vector.tensor_scalar_add(rt1[:], rt1[:], EPS)
            nc.vector.reciprocal(rt[:], rt1[:])

            # transpose rt -> row layout [1, 4096] f16
            rtT_ps = pps.tile([NCOLS, P128], f32, name="rtt", tag="rtt")
            id32 = pre.tile([P128, P128], f32, name="id32", tag="id32")
            masks.make_identity(nc, id32[:])
            nc.tensor.transpose(rtT_ps[:], rt[:], id32[:])
            rtT = pre.tile([NCOLS, P128], f16, name="rtt_sb", tag="rtt_sb")
            nc.scalar.activation(rtT[:], rtT_ps[:], AF.Identity)
            rt_row = pre.tile([1, HW], f16, name="rt_row", tag="rt_row")
            nc.sync.dma_start(
                out=rt_row[:].rearrange("o (t c) -> o t c", c=P128), in_=rtT[:]
            )

            # broadcast rt along partitions; fTn = (fT_raw + negm) * rtb (f16)
            for pc in range(8):
                rtb_ps = pps.tile([P128, 512], f32, name="rtbp", tag="rtbp", bufs=3)
                nc.tensor.matmul(
                    rtb_ps[:],
                    lhsT=ones_row[:],
                    rhs=rt_row[:, pc * 512:(pc + 1) * 512],
                    start=True,
                    stop=True,
                )
                sl = slice(pc * 512, (pc + 1) * 512)
                for c in range(C2):
                    nc.vector.scalar_tensor_tensor(
                        out=fTn[c][:, sl],
                        in0=fT_raw[c][:, sl],
                        scalar=negm[c][:],
                        in1=rtb_ps[:],
                        op0=OP.add,
                        op1=OP.mult,
                    )

            # ssqI grams AFTER the rt/rtb chain: keeps them off the
            # tensor-queue critical path that gates the loop start (ri is
            # only needed at the first evacuation, well after fTn)
            for t in range(QT):
                for kc in range(C2):
                    nc.tensor.matmul(
                        ssqI_ps[:, t:t + 1],
                        lhsT=sqI[kc][:, t * P128:(t + 1) * P128],
                        rhs=ones_col[:],
                        start=(kc == 0),
                        stop=(kc == C2 - 1),
                    )
            ri1 = pre.tile([P128, QT], f32, name="ri1", tag="ri1")
            nc.scalar.activation(ri1[:], ssqI_ps[:], AF.Sqrt)
            nc.vector.tensor_scalar_add(ri1[:], ri1[:], EPS)
            nc.vector.reciprocal(ri[:], ri1[:])

        # ---------- main loop over q tiles ----------
        rpool = ctx.enter_context(tc.tile_pool(name="rpool", bufs=2))
        r_prev = rpool.tile([P128, HW], f16, name="R", tag="R")
        nc.any.memset(r_prev[:], 0.0)

        with ExitStack() as mctx:
            mm = mctx.enter_context(tc.tile_pool(name="mm", bufs=2))
            st = mctx.enter_context(tc.tile_pool(name="st", bufs=4))
            sps = mctx.enter_context(tc.tile_pool(name="sps", bufs=1, space="PSUM"))

            ps = [sps.tile([P128, 2048], f32, name=f"ps{j}", tag=f"ps{j}") for j in range(2)]

            def emit_r_update(pend):
                """Deferred R-update: cx = wt*invw ; R = max(R, cx). Runs
                one iteration behind so DVE never waits on that tile's
                exp."""
                nonlocal r_prev
                wt_p, wse_p = pend
                invw = st.tile([P128, 1], f32, name="invw", tag="invw")
                nc.vector.reciprocal(invw[:], wse_p[:])
                cx = mm.tile([P128, HW], f16, name="cx", tag="cx")
                r_new = rpool.tile([P128, HW], f16, name="R", tag="R")
                nc.vector.tensor_scalar(
                    out=cx[:], in0=wt_p[:], scalar1=invw[:],
                    scalar2=None, op0=OP.mult,
                )
                nc.vector.tensor_max(r_new[:], cx[:], r_prev[:])
                r_prev = r_new

            pending = None
            for t in range(QT):
                tq = slice(t * P128, (t + 1) * P128)
                s16 = mm.tile([P128, HW], f16, name="s16", tag="s16")
                smx = st.tile([P128, 4], f32, name="smx", tag="smx")

                for kc in range(C2):
                    for j in range(2):
                        for h in range(4):
                            sl_p = slice((4 * j + h) * 512, (4 * j + h + 1) * 512)
                            nc.tensor.matmul(
                                ps[j][:, h * 512:(h + 1) * 512],
                                lhsT=fIc[kc][:, tq],
                                rhs=fTn[kc][:, sl_p],
                                start=(kc == 0),
                                stop=(kc == C2 - 1),
                            )

                # evacuate PSUM -> f16 SBUF on DVE, folding the ri[q]
                # scale (s16 = S_true) and a fused running max over p
                for j in range(2):
                    sl = slice(j * 2048, (j + 1) * 2048)
                    nc.vector.tensor_scalar(
                        out=s16[:, sl],
                        in0=ps[j][:],
                        scalar1=ri[:, t:t + 1],
                        scalar2=None,
                        op0=OP.mult,
                        op1=OP.max,
                        accum_out=smx[:, j:j + 1],
                    )
                smax_raw = st.tile([P128, 1], f32, name="smaxr", tag="smaxr")
                nc.vector.reduce_max(out=smax_raw[:], in_=smx[:, 0:2], axis=AX.X)

                # stats: divp = div+eps = (1-smax)/2 + eps ; rdiv = 1/divp ;
                # scl = (IS/2)*rdiv ; gam = IS - (IS/2)*rdiv
                divp = st.tile([P128, 1], f32, name="divp", tag="divp")
                rdiv = st.tile([P128, 1], f32, name="rdiv", tag="rdiv")
                scl = st.tile([P128, 1], f32, name="scl", tag="scl")
                gam = st.tile([P128, 1], f32, name="gam", tag="gam")
                nc.scalar.activation(
                    divp[:], smax_raw[:], AF.Identity,
                    bias=c_half[:], scale=-0.5,
                )
                nc.vector.reciprocal(rdiv[:], divp[:])
                nc.scalar.activation(
                    scl[:], rdiv[:], AF.Identity, scale=IS / 2.0
                )
                nc.scalar.activation(
                    gam[:], rdiv[:], AF.Identity, bias=c_is[:], scale=-IS / 2.0
                )

                # W = exp(scl*S + gam), wsum = sum_p W  (ACT)
                wt = mm.tile([P128, HW], f16, name="wt", tag="wt")
                wsum = st.tile([P128, 1], f32, name="wsum", tag="wsum")
                nc.scalar.activation(
                    wt[:], s16[:], AF.Exp, bias=gam[:], scale=scl[:],
                    accum_out=wsum[:],
                )
                wse = st.tile([P128, 1], f32, name="wse", tag="wse")
                nc.scalar.activation(wse[:], wsum[:], AF.Identity, bias=c_eps[:])

                if pending is not None:
                    emit_r_update(pending)
                pending = (wt, wse)
            emit_r_update(pending)

        # ---------- final: max over the 128 q-partitions per p ----------
        with ExitStack() as fctx:
            fin = fctx.enter_context(tc.tile_pool(name="fin", bufs=1))
            fps = fctx.enter_context(tc.tile_pool(name="fps", bufs=4, space="PSUM"))
            cxm = fin.tile([P128, NCOLS], f32, name="cxm", tag="cxm")
            for g in range(8):
                tp = fps.tile([P128, 4 * P128], f16, name="tp", tag="tp")
                for u in range(4):
                    cc = g * 4 + u
                    nc.tensor.transpose(
                        tp[:, u * P128:(u + 1) * P128],
                        r_prev[:, cc * P128:(cc + 1) * P128],
                        id16[:],
                    )
                nc.vector.reduce_max(
                    out=cxm[:, g * 4:(g + 1) * 4],
                    in_=tp[:].rearrange("p (u c) -> p u c", c=P128),
                    axis=AX.X,
                )
            nc.sync.dma_start(out=out_d[:], in_=cxm[:])

    nc.compile()
    return nc


def _get_nc():
    if "nc" not in _CACHE:
        _CACHE["nc"] = _build_nc()
    return _CACHE["nc"]


def _make_in_maps(featureT, featureI):
    featureT = np.asarray(featureT, dtype=np.float32)
    featureI = np.asarray(featureI, dtype=np.float32)
    negm = (-featureT.mean(axis=(0, 2, 3))).astype(np.float32).reshape(C2, P128, 1)
    negm = np.ascontiguousarray(negm)
    in_maps = []
    for k in range(8):
        n, h = k // 2, k % 2
        ft = np.ascontiguousarray(featureT[n].reshape(C2, P128, HW))
        fi = np.ascontiguousarray(
            featureI[n].reshape(C, HW)[:, h * QH:(h + 1) * QH].reshape(C2, P128, QH)
        )
        in_maps.append({"ft": ft, "fi": fi, "nm": negm})
    return in_maps


def _ensure_ntff_hook():
    """If tracing is requested in an environment whose antenv lacks
    axon_hooks, synthesize the module and register the ctypes NTFF hook
    so run_bass_kernel_spmd's trace path works instead of crashing."""
    import os
    import types

    try:
        from antenv import axon_hooks  # noqa: F401
        return
    except ImportError:
        pass
    try:
        mod = types.ModuleType("antenv.axon_hooks")
        _state = {"hook": None}
        mod.set_axon_ntff_profile_hook = lambda h: _state.__setitem__("hook", h)
        mod.get_axon_ntff_profile_hook = lambda: _state["hook"]
        sys.modules["antenv.axon_hooks"] = mod
        import antenv

        antenv.axon_hooks = mod
        from trn_agent_boot.trn_boot import _ntff_profile_via_ctypes

        so = "/opt/axon/libaxon_pjrt.so"
        if os.path.exists(so):
            mod.set_axon_ntff_profile_hook(_ntff_profile_via_ctypes(so))
    except Exception:
        pass


def run(featureT, featureI, trace=False):
    from concourse.bass_utils import run_bass_kernel_spmd

    _ensure_ntff_hook()
    nc = _get_nc()
    in_maps = _make_in_maps(featureT, featureI)
    res = run_bass_kernel_spmd(nc, in_maps, list(range(8)), trace=trace)
    outs = [np.asarray(res.results[k]["cxo"], dtype=np.float64) for k in range(8)]
    losses = []
    for n in range(N):
        cx = np.maximum(outs[2 * n], outs[2 * n + 1])
        losses.append(-np.log(cx.mean() + EPS))
    loss = np.float32(np.mean(losses))
    return loss, res


def kernel(featureT, featureI):
    loss, _ = run(featureT, featureI, trace=False)
    return loss


# revision 48
# speedup vs baseline: 1.2604x; 1.0005x over previous
"""CXLoss kernel for 8 Trainium2 NeuronCores (v5).

Math (per sample n):
  meanT = featureT.mean(axis=(0,2,3))                      (global over batch)
  fT = normalize(featureT[n] - meanT), fI = normalize(featureI[n] - meanT)
  S[q,p] = fI[:,q] . fT[:,p]    (C=256 contraction; p,q in [0,4096))
  raw = (1-S)/2 ; div[q] = min_p raw ; W = exp((1 - raw/(div+eps))/sigma)
  CX = W / (sum_p W + eps) ; out[p] = max_q CX ; loss = mean_n -log(mean_p out + eps)

Sharding: core k handles sample n=k//2 and half of the q axis (h=k%2).
All per-q reductions (over the full p axis) are core-local, so there is
NO cross-core communication on device (no collectives -> no cross-core
sync point; each core's execution window is its own compute only).

Host-side work is limited to input sharding/layout plus two tiny
vector-level steps that bracket the device program:
  - negm = -featureT.mean((0,2,3)) ([256] floats), passed per-core as a
    1KB side input so each core need not re-read all 4 samples (16MB)
    just to reproduce a global 256-float statistic.
  - final combine: elementwise max of core-pair outputs (8x[128,32]) and
    the -log(mean) epilogue.

Dataflow per q-tile (128 q rows x 4096 p):
  TensorE : S_psum = fIc^T @ fTn    (f16 operands; fTn is centered and
            pre-scaled by the per-p norm reciprocal rt[p])
  DVE     : evac: s16 = S_psum * ri[q] (= S_true, f16) with a fused
            max-accumulate per chunk -> smax via one tiny reduce
  ACT     : divp/scl/gam stats, W = exp(scl*s16+gam) (+wsum accum), wse
  DVE     : (one iteration behind, so DVE never waits on exp)
            invw = 1/wse ; cx = W*invw (4x) ; R = max(R, cx) (2x)
Final: transpose R 128-col blocks (TensorE) + grouped psum reduces.

Notes from HW bring-up: tensor_tensor_reduce and Pool-engine tensor ops
pass CoreSim+compile but crash TRN2 hardware here, so the evacuation
uses tensor_scalar(accum_out, op1=max) and everything heavy stays on
DVE/ACT.
"""

import sys

sys.path.insert(0, "/opt/trn_rl_repo")

import numpy as np
from contextlib import ExitStack

EPS = 1e-8
SIGMA = 0.1
IS = 1.0 / (SIGMA + EPS)  # inverse sigma

N, C, H, W = 4, 256, 64, 64
HW = H * W            # 4096 (p axis; also full q axis)
QH = HW // 2          # 2048 q per core
P128 = 128
C2 = C // P128        # 2 channel chunks
QT = QH // P128       # 16 q tiles
NCOLS = HW // P128    # 32 output columns
NEG_INF = -3.0e38

_CACHE = {}


def _build_nc():
    from concourse import bacc, mybir, masks
    from concourse import tile as tile_mod

    f32 = mybir.dt.float32
    f16 = mybir.dt.float16
    AF = mybir.ActivationFunctionType
    OP = mybir.AluOpType
    AX = mybir.AxisListType

    nc = bacc.Bacc(
        "TRN2",
        target_bir_lowering=False,
        debug=False,
        num_devices=8,
    )

    fT_d = nc.dram_tensor("ft", [C2, P128, HW], f32, kind="ExternalInput").ap()
    fI_d = nc.dram_tensor("fi", [C2, P128, QH], f32, kind="ExternalInput").ap()
    nm_d = nc.dram_tensor("nm", [C2, P128, 1], f32, kind="ExternalInput").ap()
    out_d = nc.dram_tensor("cxo", [P128, NCOLS], f32, kind="ExternalOutput").ap()

    with tile_mod.TileContext(nc) as tc, ExitStack() as ctx:
        persist = ctx.enter_context(tc.tile_pool(name="persist", bufs=1))

        # persistent matmul operands + per-q stats + constants
        fTn = [persist.tile([P128, HW], f16, name=f"ftn{c}", tag=f"ftn{c}") for c in range(C2)]
        fIc = [persist.tile([P128, QH], f16, name=f"fic{c}", tag=f"fic{c}") for c in range(C2)]
        ri = persist.tile([P128, QT], f32, name="ri", tag="ri")  # 1/(||fI||+eps)

        ones_col = persist.tile([P128, 1], f16, name="ones_col", tag="ones_col")
        ones_row = persist.tile([1, P128], f16, name="ones_row", tag="ones_row")
        id16 = persist.tile([P128, P128], f16, name="id16", tag="id16")
        negm = [persist.tile([P128, 1], f32, name=f"negm{c}", tag=f"negm{c}") for c in range(C2)]
        c_half = persist.tile([P128, 1], f32, name="c_half", tag="c_half")
        c_is = persist.tile([P128, 1], f32, name="c_is", tag="c_is")
        c_eps = persist.tile([P128, 1], f32, name="c_eps", tag="c_eps")
        nc.any.memset(ones_col[:], 1.0)
        nc.any.memset(ones_row[:], 1.0)
        nc.any.memset(c_half[:], 0.5 + EPS)
        nc.any.memset(c_is[:], IS)
        nc.any.memset(c_eps[:], EPS)
        masks.make_identity(nc, id16[:])
        for c in range(C2):
            nc.sync.dma_start(out=negm[c][:], in_=nm_d[c])

        # ---------- load + preprocess ----------
        with ExitStack() as pctx:
            pre = pctx.enter_context(tc.tile_pool(name="pre", bufs=1))
            pps = pctx.enter_context(tc.tile_pool(name="pps", bufs=1, space="PSUM"))

            fT_raw = [pre.tile([P128, HW], f32, name=f"ftraw{c}", tag=f"ftraw{c}") for c in range(C2)]
            fI_raw = [pre.tile([P128, QH], f32, name=f"firaw{c}", tag=f"firaw{c}") for c in range(C2)]
            sqT = [pre.tile([P128, HW], f16, name=f"sqt{c}", tag=f"sqt{c}") for c in range(C2)]
            sqI = [pre.tile([P128, QH], f16, name=f"sqi{c}", tag=f"sqi{c}") for c in range(C2)]

            # fT first: rt (global over p) gates fTn and hence the whole
            # loop, so its inputs must land earliest
            NJ = 4
            for c in range(C2):
                for j in range(NJ):
                    sl = slice(j * HW // NJ, (j + 1) * HW // NJ)
                    nc.sync.dma_start(out=fT_raw[c][:, sl], in_=fT_d[c][:, sl])
                    nc.scalar.activation(
                        sqT[c][:, sl], fT_raw[c][:, sl], AF.Square,
                        bias=negm[c][:], scale=1.0,
                    )
            for c in range(C2):
                for j in range(2):
                    sl = slice(j * QH // 2, (j + 1) * QH // 2)
                    nc.sync.dma_start(out=fI_raw[c][:, sl], in_=fI_d[c][:, sl])
                    nc.scalar.activation(
                        sqI[c][:, sl], fI_raw[c][:, sl], AF.Square,
                        bias=negm[c][:], scale=1.0,
                    )
                    nc.scalar.activation(
                        fIc[c][:, sl], fI_raw[c][:, sl], AF.Identity,
                        bias=negm[c][:], scale=1.0,
                    )

            # ---------- ssq columns via per-128-chunk gram with ones ----------
            # out[i, t] = sum_c sq[c, t*128+i]
            ssqT_ps = pps.tile([P128, NCOLS], f32, name="ssqt", tag="ssqt")
            ssqI_ps = pps.tile([P128, QT], f32, name="ssqi", tag="ssqi")
            for t in range(NCOLS):
                for kc in range(C2):
                    nc.tensor.matmul(
                        ssqT_ps[:, t:t + 1],
                        lhsT=sqT[kc][:, t * P128:(t + 1) * P128],
                        rhs=ones_col[:],
                        start=(kc == 0),
                        stop=(kc == C2 - 1),
                    )
            # rt = 1/(sqrt(ssqT)+eps) [128, 32]
            rt1 = pre.tile([P128, NCOLS], f32, name="rt1", tag="rt1")
            rt = pre.tile([P128, NCOLS], f32, name="rt", tag="rt")
            nc.scalar.activation(rt1[:], ssqT_ps[:], AF.Sqrt)
            nc.vector.tensor_scalar_add(rt1[:], rt1[:], EPS)
            nc.vector.reciprocal(rt[:], rt1[:])

            # transpose rt -> row layout [1, 4096] f16
            rtT_ps = pps.tile([NCOLS, P128], f32, name="rtt", tag="rtt")
            id32 = pre.tile([P128, P128], f32, name="id32", tag="id32")
            masks.make_identity(nc, id32[:])
            nc.tensor.transpose(rtT_ps[:], rt[:], id32[:])
            rtT = pre.tile([NCOLS, P128], f16, name="rtt_sb", tag="rtt_sb")
            nc.scalar.activation(rtT[:], rtT_ps[:], AF.Identity)
            rt_row = pre.tile([1, HW], f16, name="rt_row", tag="rt_row")
            nc.sync.dma_start(
                out=rt_row[:].rearrange("o (t c) -> o t c", c=P128), in_=rtT[:]
            )

            # broadcast rt along partitions; fTn = (fT_raw + negm) * rtb (f16)
            for pc in range(8):
                rtb_ps = pps.tile([P128, 512], f32, name="rtbp", tag="rtbp", bufs=3)
                nc.tensor.matmul(
                    rtb_ps[:],
                    lhsT=ones_row[:],
                    rhs=rt_row[:, pc * 512:(pc + 1) * 512],
                    start=True,
                    stop=True,
                )
                sl = slice(pc * 512, (pc + 1) * 512)
                for c in range(C2):
                    nc.vector.scalar_tensor_tensor(
                        out=fTn[c][:, sl],
                        in0=fT_raw[c][:, sl],
                        scalar=negm[c][:],
                        in1=rtb_ps[:],
                        op0=OP.add,
                        op1=OP.mult,
                    )

            # ssqI grams AFTER the rt/rtb chain: keeps them off the
            # tensor-queue critical path that gates the loop start (ri is
            # only needed at the first evacuation, well after fTn)
            for t in range(QT):
                for kc in range(C2):
                    nc.tensor.matmul(
                        ssqI_ps[:, t:t + 1],
                        lhsT=sqI[kc][:, t * P128:(t + 1) * P128],
                        rhs=ones_col[:],
                        start=(kc == 0),
                        stop=(kc == C2 - 1),
                    )
            ri1 = pre.tile([P128, QT], f32, name="ri1", tag="ri1")
            nc.scalar.activation(ri1[:], ssqI_ps[:], AF.Sqrt)
            nc.vector.tensor_scalar_add(ri1[:], ri1[:], EPS)
            nc.vector.reciprocal(ri[:], ri1[:])

        # ---------- main loop over q tiles ----------
        rpool = ctx.enter_context(tc.tile_pool(name="rpool", bufs=2))
        r_prev = rpool.tile([P128, HW], f16, name="R", tag="R")
        nc.any.memset(r_prev[:], 0.0)

        with ExitStack() as fctx:
            fin = fctx.enter_context(tc.tile_pool(name="fin", bufs=1))
            cxm = fin.tile([P128, NCOLS], f32, name="cxm", tag="cxm")
            mctx = ExitStack()
            mm = mctx.enter_context(tc.tile_pool(name="mm", bufs=2))
            st = mctx.enter_context(tc.tile_pool(name="st", bufs=4))
            sps = mctx.enter_context(tc.tile_pool(name="sps", bufs=1, space="PSUM"))

            ps = [sps.tile([P128, 2048], f32, name=f"ps{j}", tag=f"ps{j}") for j in range(2)]

            def emit_r_update(pend):
                """Deferred R-update: cx = wt*invw ; R = max(R, cx). Runs
                one iteration behind so DVE never waits on that tile's
                exp."""
                nonlocal r_prev
                wt_p, wse_p = pend
                invw = st.tile([P128, 1], f32, name="invw", tag="invw")
                nc.vector.reciprocal(invw[:], wse_p[:])
                cx = mm.tile([P128, HW], f16, name="cx", tag="cx")
                r_new = rpool.tile([P128, HW], f16, name="R", tag="R")
                nc.vector.tensor_scalar(
                    out=cx[:], in0=wt_p[:], scalar1=invw[:],
                    scalar2=None, op0=OP.mult,
                )
                nc.vector.tensor_max(r_new[:], cx[:], r_prev[:])
                r_prev = r_new

            pending = None
            for t in range(QT):
                tq = slice(t * P128, (t + 1) * P128)
                s16 = mm.tile([P128, HW], f16, name="s16", tag="s16")
                smx = st.tile([P128, 4], f32, name="smx", tag="smx")

                for kc in range(C2):
                    for j in range(2):
                        for h in range(4):
                            sl_p = slice((4 * j + h) * 512, (4 * j + h + 1) * 512)
                            nc.tensor.matmul(
                                ps[j][:, h * 512:(h + 1) * 512],
                                lhsT=fIc[kc][:, tq],
                                rhs=fTn[kc][:, sl_p],
                                start=(kc == 0),
                                stop=(kc == C2 - 1),
                            )

                # evacuate PSUM -> f16 SBUF on DVE, folding the ri[q]
                # scale (s16 = S_true) and a fused running max over p
                for j in range(2):
                    sl = slice(j * 2048, (j + 1) * 2048)
                    nc.vector.tensor_scalar(
                        out=s16[:, sl],
                        in0=ps[j][:],
                        scalar1=ri[:, t:t + 1],
                        scalar2=None,
                        op0=OP.mult,
                        op1=OP.max,
                        accum_out=smx[:, j:j + 1],
                    )
                smax_raw = st.tile([P128, 1], f32, name="smaxr", tag="smaxr")
                nc.vector.reduce_max(out=smax_raw[:], in_=smx[:, 0:2], axis=AX.X)

                # stats: divp = div+eps = (1-smax)/2 + eps ; rdiv = 1/divp ;
                # scl = (IS/2)*rdiv ; gam = IS - (IS/2)*rdiv
                divp = st.tile([P128, 1], f32, name="divp", tag="divp")
                rdiv = st.tile([P128, 1], f32, name="rdiv", tag="rdiv")
                scl = st.tile([P128, 1], f32, name="scl", tag="scl")
                gam = st.tile([P128, 1], f32, name="gam", tag="gam")
                nc.scalar.activation(
                    divp[:], smax_raw[:], AF.Identity,
                    bias=c_half[:], scale=-0.5,
                )
                nc.vector.reciprocal(rdiv[:], divp[:])
                nc.scalar.activation(
                    scl[:], rdiv[:], AF.Identity, scale=IS / 2.0
                )
                nc.scalar.activation(
                    gam[:], rdiv[:], AF.Identity, bias=c_is[:], scale=-IS / 2.0
                )

                # W = exp(scl*S + gam), wsum = sum_p W  (ACT)
                # (last tile's W/wse live in the fin pool: they are read
                # after the main-loop pools close)
                last = t == QT - 1
                wt = (fin if last else mm).tile(
                    [P128, HW], f16, name="wt", tag="wtf" if last else "wt"
                )
                wsum = st.tile([P128, 1], f32, name="wsum", tag="wsum")
                nc.scalar.activation(
                    wt[:], s16[:], AF.Exp, bias=gam[:], scale=scl[:],
                    accum_out=wsum[:],
                )
                wse = (fin if last else st).tile(
                    [P128, 1], f32, name="wse", tag="wsef" if last else "wse"
                )
                nc.scalar.activation(wse[:], wsum[:], AF.Identity, bias=c_eps[:])

                if pending is not None:
                    emit_r_update(pending)
                pending = (wt, wse)

            # final tile: R-update chunked and interleaved with the
            # transpose+reduce output stage, so the tensor engine's output
            # transposes overlap the last DVE passes instead of trailing
            # them serially (main-loop PSUM released first)
            wt_f, wse_f = pending
            invw_f = fin.tile([P128, 1], f32, name="invwf", tag="invwf")
            nc.vector.reciprocal(invw_f[:], wse_f[:])
            cx_f = fin.tile([P128, HW], f16, name="cxf", tag="cxf")
            r_fin = rpool.tile([P128, HW], f16, name="R", tag="R")
            mctx.close()
            fps = fctx.enter_context(tc.tile_pool(name="fps", bufs=4, space="PSUM"))
            for jc in range(4):
                sl = slice(jc * 1024, (jc + 1) * 1024)
                nc.vector.tensor_scalar(
                    out=cx_f[:, sl], in0=wt_f[:, sl], scalar1=invw_f[:],
                    scalar2=None, op0=OP.mult,
                )
                nc.vector.tensor_max(r_fin[:, sl], cx_f[:, sl], r_prev[:, sl])
                for g in range(2):
                    tp = fps.tile([P128, 4 * P128], f16, name="tp", tag="tp")
                    for u in range(4):
                        cc = jc * 8 + g * 4 + u
                        nc.tensor.transpose(
                            tp[:, u * P128:(u + 1) * P128],
                            r_fin[:, cc * P128:(cc + 1) * P128],
                            id16[:],
                        )
                    gc = jc * 8 + g * 4
                    nc.vector.reduce_max(
                        out=cxm[:, gc:gc + 4],
                        in_=tp[:].rearrange("p (u c) -> p u c", c=P128),
                        axis=AX.X,
                    )
            nc.sync.dma_start(out=out_d[:], in_=cxm[:])

    nc.compile()
    return nc


def _get_nc():
    if "nc" not in _CACHE:
        _CACHE["nc"] = _build_nc()
    return _CACHE["nc"]


def _make_in_maps(featureT, featureI):
    featureT = np.asarray(featureT, dtype=np.float32)
    featureI = np.asarray(featureI, dtype=np.float32)
    negm = (-featureT.mean(axis=(0, 2, 3))).astype(np.float32).reshape(C2, P128, 1)
    negm = np.ascontiguousarray(negm)
    in_maps = []
    for k in range(8):
        n, h = k // 2, k % 2
        ft = np.ascontiguousarray(featureT[n].reshape(C2, P128, HW))
        fi = np.ascontiguousarray(
            featureI[n].reshape(C, HW)[:, h * QH:(h + 1) * QH].reshape(C2, P128, QH)
        )
        in_maps.append({"ft": ft, "fi": fi, "nm": negm})
    return in_maps


def _ensure_ntff_hook():
    """If tracing is requested in an environment whose antenv lacks
    axon_hooks, synthesize the module and register the ctypes NTFF hook
    so run_bass_kernel_spmd's trace path works instead of crashing."""
    import os
    import types

    try:
        from antenv import axon_hooks  # noqa: F401
        return
    except ImportError:
        pass
    try:
        mod = types.ModuleType("antenv.axon_hooks")
        _state = {"hook": None}
        mod.set_axon_ntff_profile_hook = lambda h: _state.__setitem__("hook", h)
        mod.get_axon_ntff_profile_hook = lambda: _state["hook"]
        sys.modules["antenv.axon_hooks"] = mod
        import antenv

        antenv.axon_hooks = mod
        from trn_agent_boot.trn_boot import _ntff_profile_via_ctypes

        so = "/opt/axon/libaxon_pjrt.so"
        if os.path.exists(so):
            mod.set_axon_ntff_profile_hook(_ntff_profile_via_ctypes(so))
    except Exception:
        pass


def run(featureT, featureI, trace=False):
    from concourse.bass_utils import run_bass_kernel_spmd

    _ensure_ntff_hook()
    nc = _get_nc()
    in_maps = _make_in_maps(featureT, featureI)
    res = run_bass_kernel_spmd(nc, in_maps, list(range(8)), trace=trace)
    outs = [np.asarray(res.results[k]["cxo"], dtype=np.float64) for k in range(8)]
    losses = []
    for n in range(N):
        cx = np.maximum(outs[2 * n], outs[2 * n + 1])
        losses.append(-np.log(cx.mean() + EPS))
    loss = np.float32(np.mean(losses))
    return loss, res


def kernel(featureT, featureI):
    loss, _ = run(featureT, featureI, trace=False)
    return loss


# revision 51
# speedup vs baseline: 1.2623x; 1.0016x over previous
"""CXLoss kernel for 8 Trainium2 NeuronCores (v5).

Math (per sample n):
  meanT = featureT.mean(axis=(0,2,3))                      (global over batch)
  fT = normalize(featureT[n] - meanT), fI = normalize(featureI[n] - meanT)
  S[q,p] = fI[:,q] . fT[:,p]    (C=256 contraction; p,q in [0,4096))
  raw = (1-S)/2 ; div[q] = min_p raw ; W = exp((1 - raw/(div+eps))/sigma)
  CX = W / (sum_p W + eps) ; out[p] = max_q CX ; loss = mean_n -log(mean_p out + eps)

Sharding: core k handles sample n=k//2 and half of the q axis (h=k%2).
All per-q reductions (over the full p axis) are core-local, so there is
NO cross-core communication on device (no collectives -> no cross-core
sync point; each core's execution window is its own compute only).

Host-side work is limited to input sharding/layout plus two tiny
vector-level steps that bracket the device program:
  - negm = -featureT.mean((0,2,3)) ([256] floats), passed per-core as a
    1KB side input so each core need not re-read all 4 samples (16MB)
    just to reproduce a global 256-float statistic.
  - final combine: elementwise max of core-pair outputs (8x[128,32]) and
    the -log(mean) epilogue.

Dataflow per q-tile (128 q rows x 4096 p):
  TensorE : S_psum = fIc^T @ fTn    (f16 operands; fTn is centered and
            pre-scaled by the per-p norm reciprocal rt[p])
  DVE     : evac: s16 = S_psum * ri[q] (= S_true, f16) with a fused
            max-accumulate per chunk -> smax via one tiny reduce
  ACT     : divp/scl/gam stats, W = exp(scl*s16+gam) (+wsum accum), wse
  DVE     : (one iteration behind, so DVE never waits on exp)
            invw = 1/wse ; cx = W*invw (4x) ; R = max(R, cx) (2x)
Final: transpose R 128-col blocks (TensorE) + grouped psum reduces.

Notes from HW bring-up: tensor_tensor_reduce and Pool-engine tensor ops
pass CoreSim+compile but crash TRN2 hardware here, so the evacuation
uses tensor_scalar(accum_out, op1=max) and everything heavy stays on
DVE/ACT.
"""

import sys

sys.path.insert(0, "/opt/trn_rl_repo")

import numpy as np
from contextlib import ExitStack

EPS = 1e-8
SIGMA = 0.1
IS = 1.0 / (SIGMA + EPS)  # inverse sigma

N, C, H, W = 4, 256, 64, 64
HW = H * W            # 4096 (p axis; also full q axis)
QH = HW // 2          # 2048 q per core
P128 = 128
C2 = C // P128        # 2 channel chunks
QT = QH // P128       # 16 q tiles
NCOLS = HW // P128    # 32 output columns
NEG_INF = -3.0e38

_CACHE = {}


def _build_nc():
    from concourse import bacc, mybir, masks
    from concourse import tile as tile_mod

    f32 = mybir.dt.float32
    f16 = mybir.dt.float16
    AF = mybir.ActivationFunctionType
    OP = mybir.AluOpType
    AX = mybir.AxisListType

    nc = bacc.Bacc(
        "TRN2",
        target_bir_lowering=False,
        debug=False,
        num_devices=8,
    )

    fT_d = nc.dram_tensor("ft", [C2, P128, HW], f32, kind="ExternalInput").ap()
    fI_d = nc.dram_tensor("fi", [C2, P128, QH], f32, kind="ExternalInput").ap()
    nm_d = nc.dram_tensor("nm", [C2, P128, 1], f32, kind="ExternalInput").ap()
    out_d = nc.dram_tensor("cxo", [P128, NCOLS], f32, kind="ExternalOutput").ap()

    with tile_mod.TileContext(nc) as tc, ExitStack() as ctx:
        persist = ctx.enter_context(tc.tile_pool(name="persist", bufs=1))

        # persistent matmul operands + per-q stats + constants
        fTn = [persist.tile([P128, HW], f16, name=f"ftn{c}", tag=f"ftn{c}") for c in range(C2)]
        fIc = [persist.tile([P128, QH], f16, name=f"fic{c}", tag=f"fic{c}") for c in range(C2)]
        ri = persist.tile([P128, QT], f32, name="ri", tag="ri")  # 1/(||fI||+eps)

        ones_col = persist.tile([P128, 1], f16, name="ones_col", tag="ones_col")
        ones_row = persist.tile([1, P128], f16, name="ones_row", tag="ones_row")
        id16 = persist.tile([P128, P128], f16, name="id16", tag="id16")
        negm = [persist.tile([P128, 1], f32, name=f"negm{c}", tag=f"negm{c}") for c in range(C2)]
        c_half = persist.tile([P128, 1], f32, name="c_half", tag="c_half")
        c_is = persist.tile([P128, 1], f32, name="c_is", tag="c_is")
        c_eps = persist.tile([P128, 1], f32, name="c_eps", tag="c_eps")
        nc.any.memset(ones_col[:], 1.0)
        nc.any.memset(ones_row[:], 1.0)
        nc.any.memset(c_half[:], 0.5 + EPS)
        nc.any.memset(c_is[:], IS)
        nc.any.memset(c_eps[:], EPS)
        masks.make_identity(nc, id16[:])
        for c in range(C2):
            nc.sync.dma_start(out=negm[c][:], in_=nm_d[c])

        # ---------- load + preprocess ----------
        with ExitStack() as pctx:
            pre = pctx.enter_context(tc.tile_pool(name="pre", bufs=1))
            pps = pctx.enter_context(tc.tile_pool(name="pps", bufs=1, space="PSUM"))

            fT_raw = [pre.tile([P128, HW], f32, name=f"ftraw{c}", tag=f"ftraw{c}") for c in range(C2)]
            fI_raw = [pre.tile([P128, QH], f32, name=f"firaw{c}", tag=f"firaw{c}") for c in range(C2)]
            sqT = [pre.tile([P128, HW], f16, name=f"sqt{c}", tag=f"sqt{c}") for c in range(C2)]
            sqI = [pre.tile([P128, QH], f16, name=f"sqi{c}", tag=f"sqi{c}") for c in range(C2)]

            # fT first: rt (global over p) gates fTn and hence the whole
            # loop, so its inputs must land earliest
            NJ = 4
            for c in range(C2):
                for j in range(NJ):
                    sl = slice(j * HW // NJ, (j + 1) * HW // NJ)
                    nc.sync.dma_start(out=fT_raw[c][:, sl], in_=fT_d[c][:, sl])
                    nc.scalar.activation(
                        sqT[c][:, sl], fT_raw[c][:, sl], AF.Square,
                        bias=negm[c][:], scale=1.0,
                    )
            for c in range(C2):
                for j in range(2):
                    sl = slice(j * QH // 2, (j + 1) * QH // 2)
                    nc.sync.dma_start(out=fI_raw[c][:, sl], in_=fI_d[c][:, sl])
                    nc.scalar.activation(
                        sqI[c][:, sl], fI_raw[c][:, sl], AF.Square,
                        bias=negm[c][:], scale=1.0,
                    )
                    nc.scalar.activation(
                        fIc[c][:, sl], fI_raw[c][:, sl], AF.Identity,
                        bias=negm[c][:], scale=1.0,
                    )

            # ---------- ssq columns via per-128-chunk gram with ones ----------
            # out[i, t] = sum_c sq[c, t*128+i]
            ssqT_ps = pps.tile([P128, NCOLS], f32, name="ssqt", tag="ssqt")
            ssqI_ps = pps.tile([P128, QT], f32, name="ssqi", tag="ssqi")
            for t in range(NCOLS):
                for kc in range(C2):
                    nc.tensor.matmul(
                        ssqT_ps[:, t:t + 1],
                        lhsT=sqT[kc][:, t * P128:(t + 1) * P128],
                        rhs=ones_col[:],
                        start=(kc == 0),
                        stop=(kc == C2 - 1),
                    )
            # rt = 1/(sqrt(ssqT)+eps) [128, 32]
            rt1 = pre.tile([P128, NCOLS], f32, name="rt1", tag="rt1")
            rt = pre.tile([P128, NCOLS], f32, name="rt", tag="rt")
            nc.scalar.activation(rt1[:], ssqT_ps[:], AF.Sqrt)
            nc.vector.tensor_scalar_add(rt1[:], rt1[:], EPS)
            nc.vector.reciprocal(rt[:], rt1[:])

            # transpose rt -> row layout [1, 4096] f16
            rtT_ps = pps.tile([NCOLS, P128], f32, name="rtt", tag="rtt")
            id32 = pre.tile([P128, P128], f32, name="id32", tag="id32")
            masks.make_identity(nc, id32[:])
            nc.tensor.transpose(rtT_ps[:], rt[:], id32[:])
            rtT = pre.tile([NCOLS, P128], f16, name="rtt_sb", tag="rtt_sb")
            nc.scalar.activation(rtT[:], rtT_ps[:], AF.Identity)
            rt_row = pre.tile([1, HW], f16, name="rt_row", tag="rt_row")
            nc.sync.dma_start(
                out=rt_row[:].rearrange("o (t c) -> o t c", c=P128), in_=rtT[:]
            )

            # broadcast rt along partitions; fTn = (fT_raw + negm) * rtb (f16)
            for pc in range(8):
                rtb_ps = pps.tile([P128, 512], f32, name="rtbp", tag="rtbp", bufs=3)
                nc.tensor.matmul(
                    rtb_ps[:],
                    lhsT=ones_row[:],
                    rhs=rt_row[:, pc * 512:(pc + 1) * 512],
                    start=True,
                    stop=True,
                )
                sl = slice(pc * 512, (pc + 1) * 512)
                for c in range(C2):
                    nc.vector.scalar_tensor_tensor(
                        out=fTn[c][:, sl],
                        in0=fT_raw[c][:, sl],
                        scalar=negm[c][:],
                        in1=rtb_ps[:],
                        op0=OP.add,
                        op1=OP.mult,
                    )

            # ssqI grams AFTER the rt/rtb chain: keeps them off the
            # tensor-queue critical path that gates the loop start (ri is
            # only needed at the first evacuation, well after fTn)
            for t in range(QT):
                for kc in range(C2):
                    nc.tensor.matmul(
                        ssqI_ps[:, t:t + 1],
                        lhsT=sqI[kc][:, t * P128:(t + 1) * P128],
                        rhs=ones_col[:],
                        start=(kc == 0),
                        stop=(kc == C2 - 1),
                    )
            ri1 = pre.tile([P128, QT], f32, name="ri1", tag="ri1")
            nc.scalar.activation(ri1[:], ssqI_ps[:], AF.Sqrt)
            nc.vector.tensor_scalar_add(ri1[:], ri1[:], EPS)
            nc.vector.reciprocal(ri[:], ri1[:])

        # ---------- main loop over q tiles ----------
        rpool = ctx.enter_context(tc.tile_pool(name="rpool", bufs=2))
        r_prev = rpool.tile([P128, HW], f16, name="R", tag="R")
        nc.any.memset(r_prev[:], 0.0)

        with ExitStack() as fctx:
            fin = fctx.enter_context(tc.tile_pool(name="fin", bufs=1))
            cxm = fin.tile([P128, NCOLS], f32, name="cxm", tag="cxm")
            mctx = ExitStack()
            mm = mctx.enter_context(tc.tile_pool(name="mm", bufs=2))
            st = mctx.enter_context(tc.tile_pool(name="st", bufs=4))
            sps = mctx.enter_context(tc.tile_pool(name="sps", bufs=1, space="PSUM"))

            ps = [sps.tile([P128, 2048], f32, name=f"ps{j}", tag=f"ps{j}") for j in range(2)]

            def emit_cx_act_half(pend):
                """ACT side of the deferred R-update: the first half of
                cx = wt*invw rides the scalar engine's slack, emitted at
                the top of the next iteration so it is ready before the
                DVE tensor_max needs it."""
                wt_p, invw_p, cx = pend
                nc.scalar.activation(
                    cx[:, 0:2048], wt_p[:, 0:2048], AF.Identity,
                    scale=invw_p[:],
                )

            def emit_r_update(pend):
                """DVE side: second half of cx, then R = max(R, cx)."""
                nonlocal r_prev
                wt_p, invw_p, cx = pend
                r_new = rpool.tile([P128, HW], f16, name="R", tag="R")
                nc.vector.tensor_scalar(
                    out=cx[:, 2048:HW], in0=wt_p[:, 2048:HW],
                    scalar1=invw_p[:], scalar2=None, op0=OP.mult,
                )
                nc.vector.tensor_max(r_new[:], cx[:], r_prev[:])
                r_prev = r_new

            pending = None
            for t in range(QT):
                tq = slice(t * P128, (t + 1) * P128)
                s16 = mm.tile([P128, HW], f16, name="s16", tag="s16")
                smx = st.tile([P128, 4], f32, name="smx", tag="smx")
                if pending is not None:
                    emit_cx_act_half(pending)

                for kc in range(C2):
                    for j in range(2):
                        for h in range(4):
                            sl_p = slice((4 * j + h) * 512, (4 * j + h + 1) * 512)
                            nc.tensor.matmul(
                                ps[j][:, h * 512:(h + 1) * 512],
                                lhsT=fIc[kc][:, tq],
                                rhs=fTn[kc][:, sl_p],
                                start=(kc == 0),
                                stop=(kc == C2 - 1),
                            )

                # evacuate PSUM -> f16 SBUF on DVE, folding the ri[q]
                # scale (s16 = S_true) and a fused running max over p
                for j in range(2):
                    sl = slice(j * 2048, (j + 1) * 2048)
                    nc.vector.tensor_scalar(
                        out=s16[:, sl],
                        in0=ps[j][:],
                        scalar1=ri[:, t:t + 1],
                        scalar2=None,
                        op0=OP.mult,
                        op1=OP.max,
                        accum_out=smx[:, j:j + 1],
                    )
                smax_raw = st.tile([P128, 1], f32, name="smaxr", tag="smaxr")
                nc.vector.reduce_max(out=smax_raw[:], in_=smx[:, 0:2], axis=AX.X)

                # stats: divp = div+eps = (1-smax)/2 + eps ; rdiv = 1/divp ;
                # scl = (IS/2)*rdiv ; gam = IS - (IS/2)*rdiv
                divp = st.tile([P128, 1], f32, name="divp", tag="divp")
                rdiv = st.tile([P128, 1], f32, name="rdiv", tag="rdiv")
                scl = st.tile([P128, 1], f32, name="scl", tag="scl")
                gam = st.tile([P128, 1], f32, name="gam", tag="gam")
                nc.scalar.activation(
                    divp[:], smax_raw[:], AF.Identity,
                    bias=c_half[:], scale=-0.5,
                )
                nc.vector.reciprocal(rdiv[:], divp[:])
                nc.scalar.activation(
                    scl[:], rdiv[:], AF.Identity, scale=IS / 2.0
                )
                nc.scalar.activation(
                    gam[:], rdiv[:], AF.Identity, bias=c_is[:], scale=-IS / 2.0
                )

                # W = exp(scl*S + gam), wsum = sum_p W  (ACT)
                # (last tile's W/wse live in the fin pool: they are read
                # after the main-loop pools close)
                last = t == QT - 1
                wt = (fin if last else mm).tile(
                    [P128, HW], f16, name="wt", tag="wtf" if last else "wt"
                )
                wsum = st.tile([P128, 1], f32, name="wsum", tag="wsum")
                nc.scalar.activation(
                    wt[:], s16[:], AF.Exp, bias=gam[:], scale=scl[:],
                    accum_out=wsum[:],
                )
                wse = (fin if last else st).tile(
                    [P128, 1], f32, name="wse", tag="wsef" if last else "wse"
                )
                nc.scalar.activation(wse[:], wsum[:], AF.Identity, bias=c_eps[:])
                invw = (fin if last else st).tile(
                    [P128, 1], f32, name="invw", tag="invwf" if last else "invw"
                )
                nc.vector.reciprocal(invw[:], wse[:])

                if pending is not None:
                    emit_r_update(pending)
                pending = (
                    wt, invw,
                    (fin if last else mm).tile(
                        [P128, HW], f16, name="cx", tag="cxf" if last else "cx"
                    ),
                )

            # final tile: R-update chunked and interleaved with the
            # transpose+reduce output stage, so the tensor engine's output
            # transposes overlap the last DVE passes instead of trailing
            # them serially (main-loop PSUM released first)
            wt_f, invw_f, cx_f = pending
            r_fin = rpool.tile([P128, HW], f16, name="R", tag="R")
            mctx.close()
            fps = fctx.enter_context(tc.tile_pool(name="fps", bufs=4, space="PSUM"))
            for jc in range(4):
                sl = slice(jc * 1024, (jc + 1) * 1024)
                nc.vector.tensor_scalar(
                    out=cx_f[:, sl], in0=wt_f[:, sl], scalar1=invw_f[:],
                    scalar2=None, op0=OP.mult,
                )
                nc.vector.tensor_max(r_fin[:, sl], cx_f[:, sl], r_prev[:, sl])
                for g in range(2):
                    tp = fps.tile([P128, 4 * P128], f16, name="tp", tag="tp")
                    for u in range(4):
                        cc = jc * 8 + g * 4 + u
                        nc.tensor.transpose(
                            tp[:, u * P128:(u + 1) * P128],
                            r_fin[:, cc * P128:(cc + 1) * P128],
                            id16[:],
                        )
                    gc = jc * 8 + g * 4
                    nc.vector.reduce_max(
                        out=cxm[:, gc:gc + 4],
                        in_=tp[:].rearrange("p (u c) -> p u c", c=P128),
                        axis=AX.X,
                    )
            nc.sync.dma_start(out=out_d[:], in_=cxm[:])

    nc.compile()
    return nc


def _get_nc():
    if "nc" not in _CACHE:
        _CACHE["nc"] = _build_nc()
    return _CACHE["nc"]


def _make_in_maps(featureT, featureI):
    featureT = np.asarray(featureT, dtype=np.float32)
    featureI = np.asarray(featureI, dtype=np.float32)
    negm = (-featureT.mean(axis=(0, 2, 3))).astype(np.float32).reshape(C2, P128, 1)
    negm = np.ascontiguousarray(negm)
    in_maps = []
    for k in range(8):
        n, h = k // 2, k % 2
        ft = np.ascontiguousarray(featureT[n].reshape(C2, P128, HW))
        fi = np.ascontiguousarray(
            featureI[n].reshape(C, HW)[:, h * QH:(h + 1) * QH].reshape(C2, P128, QH)
        )
        in_maps.append({"ft": ft, "fi": fi, "nm": negm})
    return in_maps


def _ensure_ntff_hook():
    """If tracing is requested in an environment whose antenv lacks
    axon_hooks, synthesize the module and register the ctypes NTFF hook
    so run_bass_kernel_spmd's trace path works instead of crashing."""
    import os
    import types

    try:
        from antenv import axon_hooks  # noqa: F401
        return
    except ImportError:
        pass
    try:
        mod = types.ModuleType("antenv.axon_hooks")
        _state = {"hook": None}
        mod.set_axon_ntff_profile_hook = lambda h: _state.__setitem__("hook", h)
        mod.get_axon_ntff_profile_hook = lambda: _state["hook"]
        sys.modules["antenv.axon_hooks"] = mod
        import antenv

        antenv.axon_hooks = mod
        from trn_agent_boot.trn_boot import _ntff_profile_via_ctypes

        so = "/opt/axon/libaxon_pjrt.so"
        if os.path.exists(so):
            mod.set_axon_ntff_profile_hook(_ntff_profile_via_ctypes(so))
    except Exception:
        pass


def run(featureT, featureI, trace=False):
    from concourse.bass_utils import run_bass_kernel_spmd

    _ensure_ntff_hook()
    nc = _get_nc()
    in_maps = _make_in_maps(featureT, featureI)
    res = run_bass_kernel_spmd(nc, in_maps, list(range(8)), trace=trace)
    outs = [np.asarray(res.results[k]["cxo"], dtype=np.float64) for k in range(8)]
    losses = []
    for n in range(N):
        cx = np.maximum(outs[2 * n], outs[2 * n + 1])
        losses.append(-np.log(cx.mean() + EPS))
    loss = np.float32(np.mean(losses))
    return loss, res


def kernel(featureT, featureI):
    loss, _ = run(featureT, featureI, trace=False)
    return loss
